# revision 2
# baseline (speedup 1.0000x reference)
"""Haar DWT (2x2 stride-2 depthwise conv, fixed +-0.5 weights) on 8 trn2 cores.

Input  x: (8, 128, 512, 512) f32.
Output: tuple (hh, hl, lh, ll), each (8, 128, 256, 256) f32.

Sharding: pure data parallel over the batch dim — core b processes x[b].
Per-core layout: channel dim (128) -> SBUF partitions; tile over image rows.

Dataflow per tile of R rows:
  DMA in  -> ACT: x *= 0.5 (in place) -> DVE: S/D = even_rows +/- odd_rows
  -> DVE: band = S_e +/- S_o, D_e +/- D_o -> DMA out (4 bands).
"""

import numpy as np

N_CORES = 8
C = 128  # channels == SBUF partitions
H = 512
W = 512

BANDS = ("hh", "hl", "lh", "ll")  # reference return order

_CACHE = {}

# test.py can flip these before calling kernel()
TRACE = False
LAST_RESULTS = None


def _build(h, w, rows_per_tile, x_bufs=3, sd_bufs=3):
    import concourse.bacc as bacc
    import concourse.tile as tile
    import concourse.mybir as mybir

    f32 = mybir.dt.float32
    nc = bacc.Bacc("TRN2", target_bir_lowering=False, debug=False,
                   num_devices=N_CORES)

    x = nc.dram_tensor("x", [C, h, w], f32, kind="ExternalInput").ap()
    outs = {
        name: nc.dram_tensor(name, [C, h // 2, w // 2], f32,
                             kind="ExternalOutput").ap()
        for name in BANDS
    }

    R = rows_per_tile
    assert h % R == 0 and R % 4 == 0

    with tile.TileContext(nc) as tc:
        with (
            tc.tile_pool(name="xp", bufs=x_bufs) as xp,
            tc.tile_pool(name="sd", bufs=sd_bufs) as sd,
        ):
            for r0 in range(0, h, R):
                xt = xp.tile([C, R, w], f32)
                nc.sync.dma_start(out=xt[:], in_=x[:, r0:r0 + R, :])
                nc.scalar.mul(xt[:], xt[:], 0.5)

                ev = xt[:, 0::2, :]
                od = xt[:, 1::2, :]
                S = sd.tile([C, R // 2, w], f32, tag="S")
                D = sd.tile([C, R // 2, w], f32, tag="D")
                nc.vector.tensor_add(out=S[:], in0=ev, in1=od)
                nc.vector.tensor_sub(out=D[:], in0=ev, in1=od)

                # Bands overwrite the (fully consumed) x tile — saves a pool,
                # letting everything triple-buffer within SBUF.
                slots = {
                    "ll": xt[:, 0:R // 2, 0:w // 2],
                    "lh": xt[:, 0:R // 2, w // 2:w],
                    "hl": xt[:, R // 2:R, 0:w // 2],
                    "hh": xt[:, R // 2:R, w // 2:w],
                }
                pairs = {
                    "ll": (S, "add"), "lh": (S, "sub"),
                    "hl": (D, "add"), "hh": (D, "sub"),
                }
                for name in BANDS:
                    src, op = pairs[name]
                    bt = slots[name]
                    e = src[:, :, 0::2]
                    o = src[:, :, 1::2]
                    if op == "add":
                        nc.vector.tensor_add(out=bt, in0=e, in1=o)
                    else:
                        nc.vector.tensor_sub(out=bt, in0=e, in1=o)
                    nc.sync.dma_start(out=outs[name][:, r0 // 2:(r0 + R) // 2, :],
                                      in_=bt)
    nc.compile()
    return nc


def _get_nc():
    key = (H, W)
    if key not in _CACHE:
        _CACHE[key] = _build(H, W, rows_per_tile=16)
    return _CACHE[key]


def kernel(x: np.ndarray):
    global LAST_RESULTS
    from concourse.bass_utils import run_bass_kernel_spmd

    assert x.shape == (N_CORES, C, H, W), x.shape
    x = np.ascontiguousarray(x, dtype=np.float32)

    nc = _get_nc()
    in_maps = [{"x": x[b]} for b in range(N_CORES)]
    res = run_bass_kernel_spmd(nc, in_maps, core_ids=list(range(N_CORES)),
                               trace=TRACE)
    LAST_RESULTS = res

    out = tuple(
        np.stack([res.results[b][name] for b in range(N_CORES)])
        for name in BANDS
    )
    return out


# revision 3
# speedup vs baseline: 1.0877x; 1.0877x over previous
"""Haar DWT (2x2 stride-2 depthwise conv, fixed +-0.5 weights) on 8 trn2 cores.

Input  x: (8, 128, 512, 512) f32.
Output: tuple (hh, hl, lh, ll), each (8, 128, 256, 256) f32.

Sharding: pure data parallel over the batch dim — core b processes x[b].
Per-core layout: channel dim (128) -> SBUF partitions; tile over image rows.

Dataflow per tile of R rows:
  DMA in  -> ACT: x *= 0.5 (in place) -> DVE: S/D = even_rows +/- odd_rows
  -> DVE: band = S_e +/- S_o, D_e +/- D_o -> DMA out (4 bands).
"""

import numpy as np

N_CORES = 8
C = 128  # channels == SBUF partitions
H = 512
W = 512

BANDS = ("hh", "hl", "lh", "ll")  # reference return order

_CACHE = {}

# test.py can flip these before calling kernel()
TRACE = False
LAST_RESULTS = None


def _build(h, w, rows_per_tile, x_bufs=3, sd_bufs=3):
    import concourse.bacc as bacc
    import concourse.tile as tile
    import concourse.mybir as mybir

    f32 = mybir.dt.float32
    nc = bacc.Bacc("TRN2", target_bir_lowering=False, debug=False,
                   num_devices=N_CORES)

    x = nc.dram_tensor("x", [C, h, w], f32, kind="ExternalInput").ap()
    outs = {
        name: nc.dram_tensor(name, [C, h // 2, w // 2], f32,
                             kind="ExternalOutput").ap()
        for name in BANDS
    }

    R = rows_per_tile
    assert h % R == 0 and R % 4 == 0

    with tile.TileContext(nc) as tc:
        with (
            tc.tile_pool(name="xp", bufs=x_bufs) as xp,
            tc.tile_pool(name="sd", bufs=sd_bufs) as sd,
        ):
            for r0 in range(0, h, R):
                xt = xp.tile([C, R, w], f32)
                # Split the load into 4-row sub-DMAs: 8 KiB-per-partition
                # packets run ~2x faster per byte than 32 KiB ones, and
                # back-to-back issue into one tile avoids slot stalls.
                for k in range(0, R, 4):
                    nc.sync.dma_start(out=xt[:, k:k + 4, :],
                                      in_=x[:, r0 + k:r0 + k + 4, :])
                nc.scalar.mul(xt[:], xt[:], 0.5)

                ev = xt[:, 0::2, :]
                od = xt[:, 1::2, :]
                S = sd.tile([C, R // 2, w], f32, tag="S")
                D = sd.tile([C, R // 2, w], f32, tag="D")
                nc.vector.tensor_add(out=S[:], in0=ev, in1=od)
                nc.vector.tensor_sub(out=D[:], in0=ev, in1=od)

                # Bands overwrite the (fully consumed) x tile — saves a pool,
                # letting everything triple-buffer within SBUF.
                slots = {
                    "ll": xt[:, 0:R // 2, 0:w // 2],
                    "lh": xt[:, 0:R // 2, w // 2:w],
                    "hl": xt[:, R // 2:R, 0:w // 2],
                    "hh": xt[:, R // 2:R, w // 2:w],
                }
                pairs = {
                    "ll": (S, "add"), "lh": (S, "sub"),
                    "hl": (D, "add"), "hh": (D, "sub"),
                }
                for name in BANDS:
                    src, op = pairs[name]
                    bt = slots[name]
                    e = src[:, :, 0::2]
                    o = src[:, :, 1::2]
                    if op == "add":
                        nc.vector.tensor_add(out=bt, in0=e, in1=o)
                    else:
                        nc.vector.tensor_sub(out=bt, in0=e, in1=o)
                    nc.sync.dma_start(out=outs[name][:, r0 // 2:(r0 + R) // 2, :],
                                      in_=bt)
    nc.compile()
    return nc


def _get_nc():
    key = (H, W)
    if key not in _CACHE:
        _CACHE[key] = _build(H, W, rows_per_tile=16)
    return _CACHE[key]


def kernel(x: np.ndarray):
    global LAST_RESULTS
    from concourse.bass_utils import run_bass_kernel_spmd

    assert x.shape == (N_CORES, C, H, W), x.shape
    x = np.ascontiguousarray(x, dtype=np.float32)

    nc = _get_nc()
    in_maps = [{"x": x[b]} for b in range(N_CORES)]
    res = run_bass_kernel_spmd(nc, in_maps, core_ids=list(range(N_CORES)),
                               trace=TRACE)
    LAST_RESULTS = res

    out = tuple(
        np.stack([res.results[b][name] for b in range(N_CORES)])
        for name in BANDS
    )
    return out
